# revision 14
# baseline (speedup 1.0000x reference)
"""Batch-OMP dictionary learning (VQ codebook) kernel for Trainium2.

Strategy: data-parallel over the 65536 signals across 8 NeuronCores.
Per core, 64 tiles of 128 signals live on SBUF partitions. Each OMP
iteration: corr = D^T r via one PE matmul per tile, argmax via ACT
square + DVE reduce-max + max_index, atom gather via indirect DMA from
DRAM D^T, Gram/rhs entries via DVE dot products, and the per-signal
Cholesky solves run as packed [128, 64] elementwise ops (all tiles'
scalars side by side on the free axis). Coefficients are scattered into
the dense [512, N] output with single-offset indirect DMAs.
"""
import numpy as np

SP = 5
C = 64
K = 512
P = 128
NCORES = 8
N_FULL = 64 * 32 * 32 * 64 // 64  # 65536 signals
NPC = N_FULL // NCORES            # 8192 per core
GROUP = 4                         # tiles per slab DMA


def build_bass(npc=NPC, kmax=SP, do_packed=True, do_epi=True, do_par=True, do_ttr=False, do_scat=True, do_zst=True, ngroups=2):
    import concourse.bass as bass
    import concourse.mybir as mybir
    from concourse import bacc
    from concourse.tile import TileContext
    from concourse.masks import make_identity
    from concourse import bass_isa

    f32 = mybir.dt.float32
    u32 = mybir.dt.uint32
    Alu = mybir.AluOpType
    Act = mybir.ActivationFunctionType
    T = npc // P                  # signal tiles per core
    nG = T // GROUP

    nc = bacc.Bacc("TRN2", target_bir_lowering=False, debug=False)
    s_d = nc.dram_tensor("s", [C, npc], f32, kind="ExternalInput")
    d_d = nc.dram_tensor("d", [C, K], f32, kind="ExternalInput")
    dt_d = nc.dram_tensor("dt", [K, C], f32, kind="ExternalInput")
    zst_d = nc.dram_tensor("zst", [C, npc], f32, kind="ExternalOutput")
    coef_d = nc.dram_tensor("coef", [K, npc], f32, kind="ExternalOutput")
    sse_d = nc.dram_tensor("sse", [1, 1], f32, kind="ExternalOutput")

    with TileContext(nc) as tc:
        with (
            tc.tile_pool(name="const", bufs=1) as cpool,
            tc.tile_pool(name="pers", bufs=1) as pers,
            tc.tile_pool(name="work", bufs=6) as work,
            tc.tile_pool(name="ps", bufs=2, space="PSUM") as ps,
            tc.tile_pool(name="ps2", bufs=2, space="PSUM") as ps2,
        ):
            # ---- constants ----
            ident = cpool.tile([P, P], f32, tag="ident")
            make_identity(nc, ident[:])
            d_sb = cpool.tile([C, K], f32, tag="d_sb")
            nc.sync.dma_start(out=d_sb[:], in_=d_d.ap())
            iota_p = cpool.tile([P, 1], u32, tag="iota_p")
            nc.gpsimd.iota(iota_p[:], pattern=[[0, 1]], base=0, channel_multiplier=1)
            iota_pf = cpool.tile([P, 1], f32, tag="iota_pf")
            nc.vector.tensor_copy(out=iota_pf[:], in_=iota_p[:])

            # ---- zero-fill the dense coeffs output ----
            zchunk = min(2048, npc)
            zt_zero = cpool.tile([P, zchunk], f32, tag="zfill")
            nc.vector.memset(zt_zero[:], 0.0)
            coef_view = coef_d.ap().rearrange("(q p) n -> q p n", p=P)
            for q in range(K // P):
                for cc in range(npc // zchunk):
                    nc.sync.dma_start(
                        out=coef_view[q, :, cc * zchunk:(cc + 1) * zchunk],
                        in_=zt_zero[:])

            # ---- persistent per-tile tiles ----
            zT = [pers.tile([P, C], f32, name=f"zT{t}", tag=f"zT{t}") for t in range(T)]
            rT_b = [pers.tile([P, C], f32, name=f"r{t}", tag=f"r{t}") for t in range(T)]
            Wt = [[pers.tile([P, C], f32, name=f"W{k}_{t}", tag=f"W{k}_{t}") for t in range(T)]
                  for k in range(SP)]

            # ---- packed (tiles side-by-side) state, split into ngroups ----
            NG = min(ngroups, T)
            TG = T // NG
            If32 = [pers.tile([P, SP, TG], f32, name=f"If32_{g}", tag=f"If32_{g}") for g in range(NG)]
            bcol = [pers.tile([P, SP, TG], f32, name=f"bcol_{g}", tag=f"bcol_{g}") for g in range(NG)]
            gcol = [pers.tile([P, SP - 1, TG], f32, name=f"gcol_{g}", tag=f"gcol_{g}") for g in range(NG)]
            y_all = [pers.tile([P, SP, TG], f32, name=f"y_{g}", tag=f"y_{g}") for g in range(NG)]
            xs_all = [pers.tile([P, SP, TG], f32, name=f"xs_{g}", tag=f"xs_{g}") for g in range(NG)]
            nxs_all = [pers.tile([P, SP, TG], f32, name=f"nxs_{g}", tag=f"nxs_{g}") for g in range(NG)]
            Linv = [pers.tile([P, SP, TG], f32, name=f"Linv_{g}", tag=f"Linv_{g}") for g in range(NG)]
            Lmat = {(g, i, j): pers.tile([P, TG], f32, name=f"L{g}_{i}_{j}", tag=f"L{g}_{i}_{j}")
                    for g in range(NG) for i in range(1, SP) for j in range(i + 1)}
            losscol = pers.tile([P, T], f32, tag="losscol")

            # ---- stage inputs: slab chunks + zT transposes ----
            slabs = []
            for g in range(nG):
                slab = pers.tile([C, P * GROUP], f32, name=f"slab{g}", tag=f"slab{g}")
                nc.sync.dma_start(
                    out=slab[:], in_=s_d.ap()[:, g * P * GROUP:(g + 1) * P * GROUP])
                slabs.append(slab)
                for q in range(GROUP):
                    t = g * GROUP + q
                    tp = ps.tile([P, C], f32, tag="tp")
                    nc.tensor.transpose(out=tp[:], in_=slab[:, q * P:(q + 1) * P],
                                        identity=ident[:C, :C])
                    nc.scalar.activation(out=zT[t][:], in_=tp[:], func=Act.Copy)

            def dot(accum_ap, a_ap, b_ap):
                junk = work.tile([P, C], f32, tag="junk")
                nc.vector.scalar_tensor_tensor(
                    out=junk[:], in0=a_ap, scalar=1.0, in1=b_ap,
                    op0=Alu.mult, op1=Alu.mult, accum_out=accum_ap)

            # ---- OMP iterations (per-group pipelines) ----
            def selection(k, t):
                if k == 0:
                    gg, q = divmod(t, GROUP)
                    lhsT = slabs[gg][:, q * P:(q + 1) * P]
                else:
                    tp = ps.tile([C, P], f32, name="tpr", tag="tpr")
                    nc.tensor.transpose(out=tp[:], in_=rT_b[t][:],
                                        identity=ident[:])
                    rts = work.tile([C, P], f32, name="rts", tag="rts")
                    nc.scalar.activation(out=rts[:], in_=tp[:], func=Act.Copy)
                    lhsT = rts[:]
                corr = ps2.tile([P, K], f32, name="corr", tag="corr")
                nc.tensor.matmul(out=corr[:], lhsT=lhsT, rhs=d_sb[:],
                                 start=True, stop=True)
                sq = work.tile([P, K], f32, name="sq", tag="sq")
                nc.scalar.activation(out=sq[:], in_=corr[:], func=Act.Square)
                mx = work.tile([P, 1], f32, name="mx", tag="mx")
                nc.vector.tensor_reduce(out=mx[:], in_=sq[:],
                                        axis=mybir.AxisListType.X, op=Alu.max)
                idx8 = work.tile([P, 8], u32, name="idx8", tag="idx8")
                nc.vector.max_index(out=idx8[:], in_max=mx[:].to_broadcast([P, 8]),
                                    in_values=sq[:])
                g, tg = divmod(t, TG)
                nc.vector.tensor_copy(out=If32[g][:, k, tg:tg + 1], in_=idx8[:, 0:1])
                nc.gpsimd.indirect_dma_start(
                    out=Wt[k][t][:], out_offset=None, in_=dt_d.ap(),
                    in_offset=bass.IndirectOffsetOnAxis(ap=idx8[:, 0:1], axis=0))
                for j in range(k):
                    dot(gcol[g][:, j, tg:tg + 1], Wt[j][t][:], Wt[k][t][:])
                dot(bcol[g][:, k, tg:tg + 1], Wt[k][t][:], zT[t][:])

            def tt(out_ap, a_ap, b_ap, op):
                nc.vector.tensor_tensor(out=out_ap, in0=a_ap, in1=b_ap, op=op)

            def packed_solve(g, k):
                prod = work.tile([P, TG], f32, name="prod", tag="prod")
                acc = work.tile([P, TG], f32, name="acc", tag="acc")
                if k == 0:
                    nc.vector.tensor_copy(out=y_all[g][:, 0, :], in_=bcol[g][:, 0, :])
                    nc.vector.tensor_copy(out=xs_all[g][:, 0, :], in_=y_all[g][:, 0, :])
                else:
                    # forward solve L w = gcol ; w_i stored as L[k][i]
                    for i in range(k):
                        if i == 0:
                            nc.vector.tensor_copy(out=Lmat[(g, k, 0)][:], in_=gcol[g][:, 0, :])
                        else:
                            nc.vector.tensor_copy(out=acc[:], in_=gcol[g][:, i, :])
                            for j in range(i):
                                tt(prod[:], Lmat[(g, i, j)][:], Lmat[(g, k, j)][:], Alu.mult)
                                tt(acc[:], acc[:], prod[:], Alu.subtract)
                            tt(Lmat[(g, k, i)][:], acc[:], Linv[g][:, i, :], Alu.mult)
                    ssum = work.tile([P, TG], f32, name="ssum", tag="ssum")
                    tt(ssum[:], Lmat[(g, k, 0)][:], Lmat[(g, k, 0)][:], Alu.mult)
                    for i in range(1, k):
                        tt(prod[:], Lmat[(g, k, i)][:], Lmat[(g, k, i)][:], Alu.mult)
                        tt(ssum[:], ssum[:], prod[:], Alu.add)
                    nc.vector.tensor_scalar(out=ssum[:], in0=ssum[:], scalar1=-1.0,
                                            scalar2=1.0, op0=Alu.mult, op1=Alu.add)
                    nc.scalar.activation(out=Lmat[(g, k, k)][:], in_=ssum[:], func=Act.Sqrt)
                    nc.vector.reciprocal(out=Linv[g][:, k, :], in_=Lmat[(g, k, k)][:])
                    nc.vector.tensor_copy(out=acc[:], in_=bcol[g][:, k, :])
                    for j in range(k):
                        tt(prod[:], Lmat[(g, k, j)][:], y_all[g][:, j, :], Alu.mult)
                        tt(acc[:], acc[:], prod[:], Alu.subtract)
                    tt(y_all[g][:, k, :], acc[:], Linv[g][:, k, :], Alu.mult)
                    for i in range(k, -1, -1):
                        nc.vector.tensor_copy(out=acc[:], in_=y_all[g][:, i, :])
                        for j in range(i + 1, k + 1):
                            tt(prod[:], Lmat[(g, j, i)][:], xs_all[g][:, j, :], Alu.mult)
                            tt(acc[:], acc[:], prod[:], Alu.subtract)
                        if i == 0:
                            nc.vector.tensor_copy(out=xs_all[g][:, 0, :], in_=acc[:])
                        else:
                            tt(xs_all[g][:, i, :], acc[:], Linv[g][:, i, :], Alu.mult)
                for i in range(k + 1):
                    nc.vector.tensor_scalar(out=nxs_all[g][:, i, :], in0=xs_all[g][:, i, :],
                                            scalar1=-1.0, scalar2=None, op0=Alu.mult)

            def rebuild(k, t):
                g, tg = divmod(t, TG)
                nc.vector.scalar_tensor_tensor(
                    out=rT_b[t][:], in0=Wt[0][t][:],
                    scalar=nxs_all[g][:, 0, tg:tg + 1], in1=zT[t][:],
                    op0=Alu.mult, op1=Alu.add)
                for j in range(1, k + 1):
                    nc.vector.scalar_tensor_tensor(
                        out=rT_b[t][:], in0=Wt[j][t][:],
                        scalar=nxs_all[g][:, j, tg:tg + 1], in1=rT_b[t][:],
                        op0=Alu.mult, op1=Alu.add)

            coef_flat = coef_d.ap().rearrange("a (b e) -> (a b) e", e=1)

            def epilogue_group(g):
                # staging subgroups of tiles each feed one zst DMA
                sgsz = min(GROUP, TG)
                for sg in range(TG // sgsz):
                    zstT = ps.tile([C, P * sgsz], f32, name="zstT", tag="zstT")
                    for q in range(sgsz):
                        t = g * TG + sg * sgsz + q
                        tg = t - g * TG
                        zdl = work.tile([P, C], f32, name="zdl", tag="zdl")
                        nc.vector.tensor_scalar(out=zdl[:], in0=Wt[0][t][:],
                                                scalar1=xs_all[g][:, 0, tg:tg + 1],
                                                scalar2=None, op0=Alu.mult)
                        for j in range(1, SP):
                            nc.vector.scalar_tensor_tensor(
                                out=zdl[:], in0=Wt[j][t][:],
                                scalar=xs_all[g][:, j, tg:tg + 1], in1=zdl[:],
                                op0=Alu.mult, op1=Alu.add)
                        diff = work.tile([P, C], f32, name="diff", tag="diff")
                        nc.vector.tensor_tensor(out=diff[:], in0=zdl[:], in1=zT[t][:],
                                                op=Alu.subtract)
                        junk2 = work.tile([P, C], f32, name="junk2", tag="junk2")
                        nc.vector.scalar_tensor_tensor(
                            out=junk2[:], in0=diff[:], scalar=1.0, in1=diff[:],
                            op0=Alu.mult, op1=Alu.mult,
                            accum_out=losscol[:, t:t + 1])
                        zst = work.tile([P, C], f32, name="zst", tag="zst")
                        nc.vector.tensor_tensor(out=zst[:], in0=zT[t][:], in1=diff[:],
                                                op=Alu.add)
                        nc.tensor.transpose(out=zstT[:, q * P:(q + 1) * P], in_=zst[:],
                                            identity=ident[:])
                        # coeffs scatter: off_j = khat_j * npc + (t*128 + p)
                        colf = work.tile([P, 1], f32, name="colf", tag="colf")
                        nc.vector.tensor_scalar(out=colf[:], in0=iota_pf[:],
                                                scalar1=float(t * P), scalar2=None,
                                                op0=Alu.add)
                        off_f = work.tile([P, SP], f32, name="off_f", tag="off_f")
                        nc.vector.scalar_tensor_tensor(
                            out=off_f[:], in0=If32[g][:, :, tg], scalar=float(npc),
                            in1=colf[:].to_broadcast([P, SP]),
                            op0=Alu.mult, op1=Alu.add)
                        off_u = work.tile([P, SP], u32, name="off_u", tag="off_u")
                        nc.vector.tensor_copy(out=off_u[:], in_=off_f[:])
                        xs_c = work.tile([P, SP], f32, name="xs_c", tag="xs_c")
                        nc.vector.tensor_copy(out=xs_c[:], in_=xs_all[g][:, :, tg])
                        for j in range(SP):
                            nc.gpsimd.indirect_dma_start(
                                out=coef_flat,
                                out_offset=bass.IndirectOffsetOnAxis(
                                    ap=off_u[:, j:j + 1], axis=0),
                                in_=xs_c[:, j:j + 1], in_offset=None)
                    zstT_sb = work.tile([C, P * sgsz], f32, name="zstT_sb", tag="zstT_sb")
                    nc.scalar.activation(out=zstT_sb[:], in_=zstT[:C, :], func=Act.Copy)
                    base = (g * TG + sg * sgsz) * P
                    nc.sync.dma_start(
                        out=zst_d.ap()[:, base:base + P * sgsz],
                        in_=zstT_sb[:])

            for g in range(NG):
                for k in range(min(kmax, SP)):
                    for tg in range(TG):
                        selection(k, g * TG + tg)
                    if not do_packed:
                        continue
                    packed_solve(g, k)
                    if k < SP - 1:
                        for tg in range(TG):
                            rebuild(k, g * TG + tg)
                if do_epi and do_packed and kmax >= SP:
                    epilogue_group(g)

            # ---- loss: sum losscol over free dim, then over partitions ----
            lsum = cpool.tile([P, 1], f32, tag="lsum")
            nc.vector.tensor_reduce(out=lsum[:], in_=losscol[:], axis=mybir.AxisListType.X,
                                    op=Alu.add)
            ltot = cpool.tile([P, 1], f32, tag="ltot")
            if do_par:
                nc.gpsimd.partition_all_reduce(
                    out_ap=ltot[:], in_ap=lsum[:], channels=P,
                    reduce_op=bass_isa.ReduceOp.add)
            else:
                nc.vector.tensor_copy(out=ltot[:], in_=lsum[:])
            nc.sync.dma_start(out=sse_d.ap(), in_=ltot[0:1, :])
    nc.compile()
    return nc


_NC_CACHE = {}


def _get_nc(npc=NPC):
    if npc not in _NC_CACHE:
        _NC_CACHE[npc] = build_bass(npc)
    return _NC_CACHE[npc]


def kernel(z_e, dictionary, trace=False):
    from concourse.bass_utils import run_bass_kernel_spmd

    z_e = np.ascontiguousarray(np.asarray(z_e, dtype=np.float32))
    D = np.ascontiguousarray(np.asarray(dictionary, dtype=np.float32))
    # faithful to the torch .view: BCHW -> BHWC -> raw view [64, 65536]
    S = np.ascontiguousarray(z_e.transpose(0, 2, 3, 1)).reshape(C, N_FULL)
    Dt = np.ascontiguousarray(D.T)

    nc = _get_nc()
    in_maps = []
    for cid in range(NCORES):
        s_c = np.ascontiguousarray(S[:, cid * NPC:(cid + 1) * NPC])
        in_maps.append({"s": s_c, "d": D, "dt": Dt})
    res = run_bass_kernel_spmd(nc, in_maps, core_ids=list(range(NCORES)),
                               trace=trace)
    outs = res.results
    zst = np.concatenate([outs[c]["zst"] for c in range(NCORES)], axis=1)
    coef = np.concatenate([outs[c]["coef"] for c in range(NCORES)], axis=1)
    sse = sum(float(outs[c]["sse"][0, 0]) for c in range(NCORES))
    loss = np.float32(1.25 * sse / (C * N_FULL))
    z_st = zst.reshape(64, 32, 32, 64).transpose(0, 3, 1, 2)
    if trace:
        kernel.last_result = res
    return np.ascontiguousarray(z_st), loss, coef


# revision 16
# speedup vs baseline: 1.0052x; 1.0052x over previous
"""Batch-OMP dictionary learning (VQ codebook) kernel for Trainium2.

Strategy: data-parallel over the 65536 signals across 8 NeuronCores.
Per core, 64 tiles of 128 signals live on SBUF partitions. Each OMP
iteration: corr = D^T r via one PE matmul per tile, argmax via ACT
square + DVE reduce-max + max_index, atom gather via indirect DMA from
DRAM D^T, Gram/rhs entries via DVE dot products, and the per-signal
Cholesky solves run as packed [128, 64] elementwise ops (all tiles'
scalars side by side on the free axis). Coefficients are scattered into
the dense [512, N] output with single-offset indirect DMAs.
"""
import numpy as np

SP = 5
C = 64
K = 512
P = 128
NCORES = 8
N_FULL = 64 * 32 * 32 * 64 // 64  # 65536 signals
NPC = N_FULL // NCORES            # 8192 per core
GROUP = 4                         # tiles per slab DMA


def build_bass(npc=NPC, kmax=SP, do_packed=True, do_epi=True, do_par=True, do_ttr=False, do_scat=True, do_zst=True, ngroups=2):
    import concourse.bass as bass
    import concourse.mybir as mybir
    from concourse import bacc
    from concourse.tile import TileContext
    from concourse.masks import make_identity
    from concourse import bass_isa

    f32 = mybir.dt.float32
    u32 = mybir.dt.uint32
    Alu = mybir.AluOpType
    Act = mybir.ActivationFunctionType
    T = npc // P                  # signal tiles per core
    nG = T // GROUP

    nc = bacc.Bacc("TRN2", target_bir_lowering=False, debug=False)
    s_d = nc.dram_tensor("s", [C, npc], f32, kind="ExternalInput")
    d_d = nc.dram_tensor("d", [C, K], f32, kind="ExternalInput")
    dt_d = nc.dram_tensor("dt", [K, C], f32, kind="ExternalInput")
    zst_d = nc.dram_tensor("zst", [C, npc], f32, kind="ExternalOutput")
    coef_d = nc.dram_tensor("coef", [K, npc], f32, kind="ExternalOutput")
    sse_d = nc.dram_tensor("sse", [1, 1], f32, kind="ExternalOutput")

    with TileContext(nc) as tc:
        with (
            tc.tile_pool(name="const", bufs=1) as cpool,
            tc.tile_pool(name="pers", bufs=1) as pers,
            tc.tile_pool(name="work", bufs=4) as work,
            tc.tile_pool(name="ps", bufs=2, space="PSUM") as ps,
            tc.tile_pool(name="ps2", bufs=2, space="PSUM") as ps2,
        ):
            # ---- constants ----
            ident = cpool.tile([P, P], f32, tag="ident")
            make_identity(nc, ident[:])
            d_sb = cpool.tile([C, K], f32, tag="d_sb")
            nc.sync.dma_start(out=d_sb[:], in_=d_d.ap())
            iota_p = cpool.tile([P, 1], u32, tag="iota_p")
            nc.gpsimd.iota(iota_p[:], pattern=[[0, 1]], base=0, channel_multiplier=1)
            iota_pf = cpool.tile([P, 1], f32, tag="iota_pf")
            nc.vector.tensor_copy(out=iota_pf[:], in_=iota_p[:])

            # ---- zero-fill the dense coeffs output ----
            zchunk = min(2048, npc)
            zt_zero = cpool.tile([P, zchunk], f32, tag="zfill")
            nc.vector.memset(zt_zero[:], 0.0)
            coef_view = coef_d.ap().rearrange("(q p) n -> q p n", p=P)
            for q in range(K // P):
                for cc in range(npc // zchunk):
                    nc.sync.dma_start(
                        out=coef_view[q, :, cc * zchunk:(cc + 1) * zchunk],
                        in_=zt_zero[:])

            # ---- persistent per-tile tiles ----
            # ZW[t]: slot 0 = z (signal), slots 1..SP = gathered atoms, contiguous
            ZW = [pers.tile([P, SP + 1, C], f32, name=f"ZW{t}", tag=f"ZW{t}") for t in range(T)]
            rT_b = [pers.tile([P, C], f32, name=f"r{t}", tag=f"r{t}") for t in range(T)]
            zT = [ZW[t][:, 0, :] for t in range(T)]
            Wt = [[ZW[t][:, k + 1, :] for t in range(T)] for k in range(SP)]

            # ---- packed (tiles side-by-side) state, split into ngroups ----
            NG = min(ngroups, T)
            TG = T // NG
            If32 = [pers.tile([P, SP, TG], f32, name=f"If32_{g}", tag=f"If32_{g}") for g in range(NG)]
            bg = [pers.tile([P, SP, SP, TG], f32, name=f"bg_{g}", tag=f"bg_{g}") for g in range(NG)]
            y_all = [pers.tile([P, SP, TG], f32, name=f"y_{g}", tag=f"y_{g}") for g in range(NG)]
            xs_all = [pers.tile([P, SP, TG], f32, name=f"xs_{g}", tag=f"xs_{g}") for g in range(NG)]
            nxs_all = [pers.tile([P, SP, TG], f32, name=f"nxs_{g}", tag=f"nxs_{g}") for g in range(NG)]
            Linv = [pers.tile([P, SP, TG], f32, name=f"Linv_{g}", tag=f"Linv_{g}") for g in range(NG)]
            Lmat = {(g, i, j): pers.tile([P, TG], f32, name=f"L{g}_{i}_{j}", tag=f"L{g}_{i}_{j}")
                    for g in range(NG) for i in range(1, SP) for j in range(i + 1)}
            losscol = pers.tile([P, T], f32, tag="losscol")

            # ---- stage inputs: slab chunks + zT transposes ----
            slabs = []
            for g in range(nG):
                slab = pers.tile([C, P * GROUP], f32, name=f"slab{g}", tag=f"slab{g}")
                nc.sync.dma_start(
                    out=slab[:], in_=s_d.ap()[:, g * P * GROUP:(g + 1) * P * GROUP])
                slabs.append(slab)
                for q in range(GROUP):
                    t = g * GROUP + q
                    tp = ps.tile([P, C], f32, tag="tp")
                    nc.tensor.transpose(out=tp[:], in_=slab[:, q * P:(q + 1) * P],
                                        identity=ident[:C, :C])
                    nc.scalar.activation(out=zT[t], in_=tp[:], func=Act.Copy)

            def dot(accum_ap, a_ap, b_ap):
                junk = work.tile([P, C], f32, tag="junk")
                nc.vector.scalar_tensor_tensor(
                    out=junk[:], in0=a_ap, scalar=1.0, in1=b_ap,
                    op0=Alu.mult, op1=Alu.mult, accum_out=accum_ap)

            # ---- OMP iterations (per-group pipelines) ----
            def selection(k, t):
                if k == 0:
                    gg, q = divmod(t, GROUP)
                    lhsT = slabs[gg][:, q * P:(q + 1) * P]
                else:
                    tp = ps.tile([C, P], f32, name="tpr", tag="tpr")
                    nc.tensor.transpose(out=tp[:], in_=rT_b[t][:],
                                        identity=ident[:])
                    rts = work.tile([C, P], f32, name="rts", tag="rts")
                    nc.scalar.activation(out=rts[:], in_=tp[:], func=Act.Copy)
                    lhsT = rts[:]
                corr = ps2.tile([P, K], f32, name="corr", tag="corr")
                nc.tensor.matmul(out=corr[:], lhsT=lhsT, rhs=d_sb[:],
                                 start=True, stop=True)
                sq = work.tile([P, K], f32, name="sq", tag="sq")
                nc.scalar.activation(out=sq[:], in_=corr[:], func=Act.Square)
                mx = work.tile([P, 1], f32, name="mx", tag="mx")
                nc.vector.tensor_reduce(out=mx[:], in_=sq[:],
                                        axis=mybir.AxisListType.X, op=Alu.max)
                idx8 = work.tile([P, 8], u32, name="idx8", tag="idx8")
                nc.vector.max_index(out=idx8[:], in_max=mx[:].to_broadcast([P, 8]),
                                    in_values=sq[:])
                g, tg = divmod(t, TG)
                nc.vector.tensor_copy(out=If32[g][:, k, tg:tg + 1], in_=idx8[:, 0:1])
                nc.gpsimd.indirect_dma_start(
                    out=Wt[k][t], out_offset=None, in_=dt_d.ap(),
                    in_offset=bass.IndirectOffsetOnAxis(ap=idx8[:, 0:1], axis=0))
                for j in range(k):
                    pass  # dots batched below
                # batched dots: products of W_k against [z, W_0..W_{k-1}], one
                # multiply + one strided reduce; elem 0 -> b_k, 1+j -> g_j
                nslots = k + 1
                prodw = work.tile([P, (SP + 1) * C], f32, name="prodw", tag="prodw", bufs=2)
                nc.vector.tensor_tensor(
                    out=prodw[:, :nslots * C].rearrange("p (j c) -> p j c", j=nslots),
                    in0=ZW[t][:, 0:nslots, :],
                    in1=ZW[t][:, k + 1:k + 2, :].to_broadcast([P, nslots, C]),
                    op=Alu.mult)
                nc.vector.tensor_reduce(
                    out=bg[g][:, k, 0:nslots, tg],
                    in_=prodw[:, :nslots * C].rearrange("p (j c) -> p j c", j=nslots),
                    axis=mybir.AxisListType.X, op=Alu.add)

            def tt(out_ap, a_ap, b_ap, op):
                nc.vector.tensor_tensor(out=out_ap, in0=a_ap, in1=b_ap, op=op)

            def packed_solve(g, k):
                prod = work.tile([P, TG], f32, name="prod", tag="prod")
                acc = work.tile([P, TG], f32, name="acc", tag="acc")
                if k == 0:
                    nc.vector.tensor_copy(out=y_all[g][:, 0, :], in_=bg[g][:, 0, 0, :])
                    nc.vector.tensor_copy(out=xs_all[g][:, 0, :], in_=y_all[g][:, 0, :])
                else:
                    # forward solve L w = gcol ; w_i stored as L[k][i]
                    for i in range(k):
                        if i == 0:
                            nc.vector.tensor_copy(out=Lmat[(g, k, 0)][:], in_=bg[g][:, k, 1, :])
                        else:
                            nc.vector.tensor_copy(out=acc[:], in_=bg[g][:, k, 1 + i, :])
                            for j in range(i):
                                tt(prod[:], Lmat[(g, i, j)][:], Lmat[(g, k, j)][:], Alu.mult)
                                tt(acc[:], acc[:], prod[:], Alu.subtract)
                            tt(Lmat[(g, k, i)][:], acc[:], Linv[g][:, i, :], Alu.mult)
                    ssum = work.tile([P, TG], f32, name="ssum", tag="ssum")
                    tt(ssum[:], Lmat[(g, k, 0)][:], Lmat[(g, k, 0)][:], Alu.mult)
                    for i in range(1, k):
                        tt(prod[:], Lmat[(g, k, i)][:], Lmat[(g, k, i)][:], Alu.mult)
                        tt(ssum[:], ssum[:], prod[:], Alu.add)
                    nc.vector.tensor_scalar(out=ssum[:], in0=ssum[:], scalar1=-1.0,
                                            scalar2=1.0, op0=Alu.mult, op1=Alu.add)
                    nc.scalar.activation(out=Lmat[(g, k, k)][:], in_=ssum[:], func=Act.Sqrt)
                    nc.vector.reciprocal(out=Linv[g][:, k, :], in_=Lmat[(g, k, k)][:])
                    nc.vector.tensor_copy(out=acc[:], in_=bg[g][:, k, 0, :])
                    for j in range(k):
                        tt(prod[:], Lmat[(g, k, j)][:], y_all[g][:, j, :], Alu.mult)
                        tt(acc[:], acc[:], prod[:], Alu.subtract)
                    tt(y_all[g][:, k, :], acc[:], Linv[g][:, k, :], Alu.mult)
                    for i in range(k, -1, -1):
                        nc.vector.tensor_copy(out=acc[:], in_=y_all[g][:, i, :])
                        for j in range(i + 1, k + 1):
                            tt(prod[:], Lmat[(g, j, i)][:], xs_all[g][:, j, :], Alu.mult)
                            tt(acc[:], acc[:], prod[:], Alu.subtract)
                        if i == 0:
                            nc.vector.tensor_copy(out=xs_all[g][:, 0, :], in_=acc[:])
                        else:
                            tt(xs_all[g][:, i, :], acc[:], Linv[g][:, i, :], Alu.mult)
                for i in range(k + 1):
                    nc.vector.tensor_scalar(out=nxs_all[g][:, i, :], in0=xs_all[g][:, i, :],
                                            scalar1=-1.0, scalar2=None, op0=Alu.mult)

            def rebuild(k, t):
                g, tg = divmod(t, TG)
                nc.vector.scalar_tensor_tensor(
                    out=rT_b[t][:], in0=Wt[0][t],
                    scalar=nxs_all[g][:, 0, tg:tg + 1], in1=zT[t],
                    op0=Alu.mult, op1=Alu.add)
                for j in range(1, k + 1):
                    nc.vector.scalar_tensor_tensor(
                        out=rT_b[t][:], in0=Wt[j][t],
                        scalar=nxs_all[g][:, j, tg:tg + 1], in1=rT_b[t][:],
                        op0=Alu.mult, op1=Alu.add)

            coef_flat = coef_d.ap().rearrange("a (b e) -> (a b) e", e=1)

            def epilogue_group(g):
                # staging subgroups of tiles each feed one zst DMA
                sgsz = min(GROUP, TG)
                for sg in range(TG // sgsz):
                    zstT = ps.tile([C, P * sgsz], f32, name="zstT", tag="zstT")
                    for q in range(sgsz):
                        t = g * TG + sg * sgsz + q
                        tg = t - g * TG
                        zdl = work.tile([P, C], f32, name="zdl", tag="zdl")
                        nc.vector.tensor_scalar(out=zdl[:], in0=Wt[0][t],
                                                scalar1=xs_all[g][:, 0, tg:tg + 1],
                                                scalar2=None, op0=Alu.mult)
                        for j in range(1, SP):
                            nc.vector.scalar_tensor_tensor(
                                out=zdl[:], in0=Wt[j][t],
                                scalar=xs_all[g][:, j, tg:tg + 1], in1=zdl[:],
                                op0=Alu.mult, op1=Alu.add)
                        diff = work.tile([P, C], f32, name="diff", tag="diff")
                        nc.vector.tensor_tensor(out=diff[:], in0=zdl[:], in1=zT[t],
                                                op=Alu.subtract)
                        junk2 = work.tile([P, C], f32, name="junk2", tag="junk2")
                        nc.vector.scalar_tensor_tensor(
                            out=junk2[:], in0=diff[:], scalar=1.0, in1=diff[:],
                            op0=Alu.mult, op1=Alu.mult,
                            accum_out=losscol[:, t:t + 1])
                        zst = work.tile([P, C], f32, name="zst", tag="zst")
                        nc.vector.tensor_tensor(out=zst[:], in0=zT[t], in1=diff[:],
                                                op=Alu.add)
                        nc.tensor.transpose(out=zstT[:, q * P:(q + 1) * P], in_=zst[:],
                                            identity=ident[:])
                        # coeffs scatter: off_j = khat_j * npc + (t*128 + p)
                        colf = work.tile([P, 1], f32, name="colf", tag="colf")
                        nc.vector.tensor_scalar(out=colf[:], in0=iota_pf[:],
                                                scalar1=float(t * P), scalar2=None,
                                                op0=Alu.add)
                        off_f = work.tile([P, SP], f32, name="off_f", tag="off_f")
                        nc.vector.scalar_tensor_tensor(
                            out=off_f[:], in0=If32[g][:, :, tg], scalar=float(npc),
                            in1=colf[:].to_broadcast([P, SP]),
                            op0=Alu.mult, op1=Alu.add)
                        off_u = work.tile([P, SP], u32, name="off_u", tag="off_u")
                        nc.vector.tensor_copy(out=off_u[:], in_=off_f[:])
                        xs_c = work.tile([P, SP], f32, name="xs_c", tag="xs_c")
                        nc.vector.tensor_copy(out=xs_c[:], in_=xs_all[g][:, :, tg])
                        for j in range(SP):
                            nc.gpsimd.indirect_dma_start(
                                out=coef_flat,
                                out_offset=bass.IndirectOffsetOnAxis(
                                    ap=off_u[:, j:j + 1], axis=0),
                                in_=xs_c[:, j:j + 1], in_offset=None)
                    zstT_sb = work.tile([C, P * sgsz], f32, name="zstT_sb", tag="zstT_sb")
                    nc.scalar.activation(out=zstT_sb[:], in_=zstT[:C, :], func=Act.Copy)
                    base = (g * TG + sg * sgsz) * P
                    nc.sync.dma_start(
                        out=zst_d.ap()[:, base:base + P * sgsz],
                        in_=zstT_sb[:])

            for g in range(NG):
                for k in range(min(kmax, SP)):
                    for tg in range(TG):
                        selection(k, g * TG + tg)
                    if not do_packed:
                        continue
                    packed_solve(g, k)
                    if k < SP - 1:
                        for tg in range(TG):
                            rebuild(k, g * TG + tg)
                if do_epi and do_packed and kmax >= SP:
                    epilogue_group(g)

            # ---- loss: sum losscol over free dim, then over partitions ----
            lsum = cpool.tile([P, 1], f32, tag="lsum")
            nc.vector.tensor_reduce(out=lsum[:], in_=losscol[:], axis=mybir.AxisListType.X,
                                    op=Alu.add)
            ltot = cpool.tile([P, 1], f32, tag="ltot")
            if do_par:
                nc.gpsimd.partition_all_reduce(
                    out_ap=ltot[:], in_ap=lsum[:], channels=P,
                    reduce_op=bass_isa.ReduceOp.add)
            else:
                nc.vector.tensor_copy(out=ltot[:], in_=lsum[:])
            nc.sync.dma_start(out=sse_d.ap(), in_=ltot[0:1, :])
    nc.compile()
    return nc


_NC_CACHE = {}


def _get_nc(npc=NPC):
    if npc not in _NC_CACHE:
        _NC_CACHE[npc] = build_bass(npc)
    return _NC_CACHE[npc]


def kernel(z_e, dictionary, trace=False):
    from concourse.bass_utils import run_bass_kernel_spmd

    z_e = np.ascontiguousarray(np.asarray(z_e, dtype=np.float32))
    D = np.ascontiguousarray(np.asarray(dictionary, dtype=np.float32))
    # faithful to the torch .view: BCHW -> BHWC -> raw view [64, 65536]
    S = np.ascontiguousarray(z_e.transpose(0, 2, 3, 1)).reshape(C, N_FULL)
    Dt = np.ascontiguousarray(D.T)

    nc = _get_nc()
    in_maps = []
    for cid in range(NCORES):
        s_c = np.ascontiguousarray(S[:, cid * NPC:(cid + 1) * NPC])
        in_maps.append({"s": s_c, "d": D, "dt": Dt})
    res = run_bass_kernel_spmd(nc, in_maps, core_ids=list(range(NCORES)),
                               trace=trace)
    outs = res.results
    zst = np.concatenate([outs[c]["zst"] for c in range(NCORES)], axis=1)
    coef = np.concatenate([outs[c]["coef"] for c in range(NCORES)], axis=1)
    sse = sum(float(outs[c]["sse"][0, 0]) for c in range(NCORES))
    loss = np.float32(1.25 * sse / (C * N_FULL))
    z_st = zst.reshape(64, 32, 32, 64).transpose(0, 3, 1, 2)
    if trace:
        kernel.last_result = res
    return np.ascontiguousarray(z_st), loss, coef
